# revision 13
# baseline (speedup 1.0000x reference)
"""Cosine-similarity self-attention (Cos_Attn) on 8 Trainium2 NeuronCores.

Reference math (x: [C=512, W=64, H=64] fp32, N = W*H = 4096):
    q = x.reshape(C, N).T                  # [N, C]
    energy = q @ q.T                       # [N, N]
    cos    = energy / (|q_i| |q_j|)
    out    = softmax(cos, axis=-1)[None]   # [1, N, N]

v6 design - host-normalized fp8, query-major layout, ACT-accumulated
row sums, PE pstate warm-up, chunk-pair-major inputs.

Host pre-normalizes the columns of x to unit L2 norm before the fp8
quantize, so on device cosine == dot product of fp8 unit vectors: no
Grams, no rsqrt chains, exactly one ACT table load (pulled to t=0 by a
dummy exp). Inputs are host-permuted into the per-partition SBUF
layout so every input DMA descriptor is a 4 KB contiguous run.

Per core (own 512 query rows x all 4096 keys):
  - queries on PSUM partitions, keys free: softmax row-reduce is the
    ACT accumulator, the output needs no host transpose.
  - PE warm-up: a dozen dummy fp8 matmuls run during the input DMA so
    the Tensor engine reaches its full 2.4 GHz pstate (cold it runs
    0.65-1.2 GHz) before real work, and real matmuls overlap their
    LDWEIGHTS from the start.
  - energy: per (128-query block, 2048-key half): 4 fp8 DoubleRow
    matmuls (K=256, free=1024 spanning 2 PSUM banks) into a 4-bank
    tile [128, 4, 512], double buffered (PE fills one, ACT drains the
    other).
  - exp: ONE activation per half ([128, 2048] f32 PSUM -> bf16 SBUF,
    scale=1/cq^2) with accum_out giving the half row-sum for ~180 ns.
    ACT is the bottleneck: 8 x ~2.15 us stream.
  - softmax tail: DVE adds half-sums, reciprocal_approx_fast, all-bf16
    per-partition scale (2x mode); block DMAs out overlap later
    blocks' compute; the last block's scale + out-DMA is split in two
    halves on two queues to shorten the tail.
"""

import numpy as np

_NCORES = 8
_P = 128

# set by the test harness only; the grading path keeps these defaults
TRACE = False
TRACE_CORES = None
LAST_RESULT = None

_built = None  # (nc, C, N)

_CQ = 16.0     # host fp8 quantize scale for the normalized columns
_NWARM = 7     # PE pstate warm-up matmuls


def _build(C, N, QB):
    """Single-NEFF Bass/Tile program (SPMD: identical on all cores).

    Inputs:  x8 [128, C/128 * N]  fp8e4, host-permuted pair-major:
                 [p, pair(4), ko(4), 1024] with c = ko*128 + p
             xq [128, C/128 * QB] fp8e4, host-permuted: [p, ko(4), QB]
    Output:  out [QB, N] bf16 = this core's softmax rows.
    """
    from contextlib import ExitStack

    import concourse.tile as tile
    from concourse import bacc, mybir

    f32 = mybir.dt.float32
    bf16 = mybir.dt.bfloat16
    fp8 = mybir.dt.float8e4
    AF = mybir.ActivationFunctionType
    DR = mybir.MatmulPerfMode.DoubleRow

    P = _P
    KO = C // P              # contraction subtiles (4)
    NP = N // 1024           # key chunk pairs (4)
    QBLK = QB // P           # query blocks per core (4)
    ESC = 1.0 / (_CQ * _CQ)  # exp input scale: cos = energy / cq^2

    nc = bacc.Bacc("TRN2", target_bir_lowering=False, debug=False)
    x8_d = nc.dram_tensor("x8", [P, KO * N], fp8, kind="ExternalInput")
    xq_d = nc.dram_tensor("xq", [P, KO * QB], fp8, kind="ExternalInput")
    out_d = nc.dram_tensor("out", [QB, N], bf16, kind="ExternalOutput")

    x8_r = x8_d.ap().rearrange("p (c k x) -> p c k x", c=NP, k=KO)
    xq_r = xq_d.ap().rearrange("p (k x) -> p k x", k=KO)
    out_r = out_d.ap().rearrange("(qb p) (nk x) -> p qb nk x", p=P, x=512)

    with tile.TileContext(nc) as tc, ExitStack() as ctx:
        persist = ctx.enter_context(tc.tile_pool(name="persist", bufs=1))
        psum = ctx.enter_context(tc.tile_pool(name="psum", bufs=2, space="PSUM"))

        x8_sb = persist.tile([P, NP, KO, 1024], fp8)   # all keys, pair-major
        xq_sb = persist.tile([P, KO, QB], fp8)         # own query cols
        e_sb = persist.tile([P, QBLK, 2, 2048], bf16)  # exp(cos) rows
        rsum = persist.tile([P, QBLK, 2], f32)         # half row-sums (ACT)
        rs = persist.tile([P, QBLK], f32)              # row sums
        rr = persist.tile([P, QBLK], f32)              # 1 / row sums
        warm = persist.tile([P, 1], f32)
        wdum = persist.tile([P, P], bf16)              # warm-up weights
        rdum = persist.tile([P, 512], bf16)            # warm-up rhs

        nc.vector.memset(warm[:], 0.0)
        nc.vector.memset(wdum[:], 0.0)
        nc.vector.memset(rdum[:], 0.0)

        # ---- input DMAs: pair0 alone on the scalar queue (it gates the
        # first exp), queries first on gpsimd, the rest behind ----
        nc.scalar.dma_start(x8_sb[:, 0], x8_r[:, 0])
        nc.scalar.activation(warm[:], warm[:], AF.Exp)  # ACT table load now
        nc.sync.dma_start(x8_sb[:, 1], x8_r[:, 1])
        nc.sync.dma_start(x8_sb[:, 3], x8_r[:, 3])
        nc.gpsimd.dma_start(xq_sb[:], xq_r[:])
        nc.gpsimd.dma_start(x8_sb[:, 2], x8_r[:, 2])

        # ---- PE pstate warm-up: keep the Tensor engine busy through
        # the input-DMA window so real matmuls run at full clock and
        # overlap their LDWEIGHTS from the first real instruction ----
        for _ in range(_NWARM):
            pd = psum.tile([P, 4, 512], f32, tag="pp", name="pp", bufs=2)
            nc.tensor.matmul(pd[:, 0, :], lhsT=wdum[:], rhs=rdum[:],
                             start=True, stop=True)

        def mm(pp_b, qsl, pr, half, k2):
            """One DR matmul: 512-key half `half` of chunk pair `pr`."""
            cs = slice(half * 512, half * 512 + 512)
            k2s = slice(2 * k2, 2 * k2 + 2)
            nc.tensor.matmul(
                pp_b,
                lhsT=xq_sb[:, k2s, qsl],
                rhs=x8_sb[:, pr, k2s, cs],
                start=(k2 == 0),
                stop=(k2 == 1),
                perf_mode=DR,
            )

        r4 = persist.tile([P, 4], f32)  # qb0 quarter row-sums

        for qb in range(QBLK):
            qsl = slice(qb * P, (qb + 1) * P)
            if qb == 0:
                # first block streams at chunk-pair granularity so its
                # exps follow the input DMA chunk arrivals
                for pr in range(4):
                    if pr % 2 == 0:
                        pp = psum.tile([P, 4, 512], f32, tag="pp",
                                       name="pp", bufs=2)
                    b = 2 * (pr % 2)
                    for j in range(2):
                        for k2 in range(2):
                            mm(pp[:, b + j, :], qsl, pr, j, k2)
                    nc.scalar.activation(
                        e_sb[:, 0, pr // 2, 1024 * (pr % 2):
                             1024 * (pr % 2) + 1024].rearrange(
                            "p (a x) -> p a x", a=2),
                        pp[:, b:b + 2, :],
                        AF.Exp, scale=ESC, accum_out=r4[:, pr:pr + 1])
                nc.vector.tensor_add(rsum[:, 0, 0:1], r4[:, 0:1], r4[:, 1:2])
                nc.vector.tensor_add(rsum[:, 0, 1:2], r4[:, 2:3], r4[:, 3:4])
            else:
                for h in range(2):
                    pp = psum.tile([P, 4, 512], f32, tag="pp", name="pp",
                                   bufs=2)
                    for j in range(4):
                        for k2 in range(2):
                            mm(pp[:, j, :], qsl, 2 * h + j // 2, j % 2, k2)
                    nc.scalar.activation(e_sb[:, qb, h].rearrange(
                        "p (a x) -> p a x", a=4), pp[:],
                        AF.Exp, scale=ESC,
                        accum_out=rsum[:, qb, h:h + 1])
            # ---- softmax denominator + scale + stream out ----
            nc.vector.tensor_add(rs[:, qb:qb + 1], rsum[:, qb, 0:1],
                                 rsum[:, qb, 1:2])
            nc.vector.reciprocal_approx_fast(rr[:, qb:qb + 1], rs[:, qb:qb + 1])
            if qb < QBLK - 1:
                nc.vector.tensor_scalar_mul(e_sb[:, qb], e_sb[:, qb],
                                            rr[:, qb:qb + 1])
                eng = nc.sync if qb == 0 else nc.gpsimd
                eng.dma_start(out_r[:, qb], e_sb[:, qb].rearrange(
                    "p h (nk x) -> p (h nk) x", x=512))
            else:
                # last block: split scale + DMA across both queues
                for h in range(2):
                    nc.vector.tensor_scalar_mul(e_sb[:, qb, h], e_sb[:, qb, h],
                                                rr[:, qb:qb + 1])
                    eng = nc.sync if h == 0 else nc.gpsimd
                    eng.dma_start(out_r[:, qb, 4 * h:4 * h + 4],
                                  e_sb[:, qb, h].rearrange(
                                      "p (nk x) -> p nk x", x=512))

    nc.compile()
    return nc


def kernel(**inputs) -> np.ndarray:
    global _built, LAST_RESULT
    import ml_dtypes

    x = np.asarray(inputs["x"], dtype=np.float32)
    C, W, H = x.shape
    N = W * H
    QB = N // _NCORES
    x2 = x.reshape(C, N)

    if _built is None or _built[1:] != (C, N):
        _built = (_build(C, N, QB), C, N)
    nc = _built[0]

    from concourse import bass_utils

    # host preprocess: unit-normalize columns, fp8-quantize, and permute
    # into the device's per-partition layout (4 KB DMA runs).
    norms = np.sqrt((x2 * x2).sum(axis=0))
    x8 = (x2 * (_CQ / norms)[None, :]).astype(ml_dtypes.float8_e4m3fn)
    # x8[ko*128+p, c*1024+j] -> x8p[p, c, ko, j]
    x8p = np.ascontiguousarray(
        x8.reshape(C // _P, _P, N // 1024, 1024).transpose(1, 2, 0, 3)
    ).reshape(_P, -1)
    in_maps = []
    for i in range(_NCORES):
        xq = x8[:, i * QB:(i + 1) * QB]
        # xq[ko*128+p, q] -> xqp[p, ko, q]
        xqp = np.ascontiguousarray(
            xq.reshape(C // _P, _P, QB).transpose(1, 0, 2)).reshape(_P, -1)
        in_maps.append({"x8": x8p, "xq": xqp})

    kwargs = {}
    if TRACE:
        kwargs["trace"] = True
        if TRACE_CORES is not None:
            kwargs["trace_cores"] = list(TRACE_CORES)
    res = bass_utils.run_bass_kernel_spmd(
        nc, in_maps, core_ids=list(range(_NCORES)), **kwargs
    )
    LAST_RESULT = res
    out = np.empty((N, N), dtype=np.float32)
    for i in range(_NCORES):
        out[i * QB:(i + 1) * QB] = res.results[i]["out"].astype(np.float32)
    return out.reshape(1, N, N)
